# revision 1
# baseline (speedup 1.0000x reference)
"""Fused 2-layer GCN forward (nn_Net_SSL_38740605010537) on 8 Trainium2
NeuronCores - single launch, self-contained.

out = log_softmax(A @ relu(A @ (x@W1) + b1) @ W2 + b2),
A = D^-1/2 (Adj+I) D^-1/2 with D^-1/2 folded into per-node scales.

Per core: x@W1 on PE -> 4 per-bucket AllGathers -> windowed dma_gather +
DVE segment-reduce + dma_scatter_add on 4 SWDGE queues -> per-strip
epilogue adding the dense self-loop term (emitted inline so it overlaps
the aggregation) -> 4 AllGathers -> second aggregation -> classifier +
batched log_softmax.  The gather/scatter schedule is built on the host;
the +1 self-loop is applied densely, never gathered.  Scatter
sub-schedules keep target rows disjoint and are fenced by dummy-read DMA
barriers, because concurrent CCE read-modify-write to one HBM row loses
updates (measured on hardware).
"""
import hashlib

import numpy as np

N_NODES, N_EDGES = 100000, 1600000
F_IN, HID, N_CLS = 256, 64, 40
NC, PER_CORE = 8, 12500
NB = 4
SJ, STRIP = 3125, 3200
CHUNK = 4 * STRIP            # 12800
BUCKET_ROWS = 8 * STRIP      # 25600
NG = 25                      # base groups per strip
YROWS = STRIP + 128          # yacc strip tensor rows
TRASH = 3200
DUMMY = 3125
WCOLS = 48
GCALL = 8
SCALL = 16
P = 128


def _wrap16_blocks(blocks):
    out = []
    for fl in blocks:
        n = fl.shape[0]
        w = fl.reshape(n // 16, 16).T
        out.append(np.tile(w, (8, 1)))
    return np.concatenate(out, axis=1).astype(np.int16)


def build_schedule(edge_index):
    src = np.asarray(edge_index[0], dtype=np.int64)
    dst = np.asarray(edge_index[1], dtype=np.int64)
    deg = np.bincount(dst, minlength=N_NODES).astype(np.float64) + 1.0
    dinv = (deg ** -0.5).astype(np.float32)

    co = src // PER_CORE
    ii = src % PER_CORE
    b_e = ii // SJ
    sloc_e = co * STRIP + (ii % SJ)
    cd_e = dst // PER_CORE
    jj = dst % PER_CORE
    sd_e = jj // SJ
    jloc_e = jj % SJ

    cnt = np.zeros((NC, NB, 4, SJ), np.int64)
    np.add.at(cnt, (cd_e, b_e, sd_e, jloc_e), 1)

    # ---- packing per (b, sd): main (no requeue) + tail
    Dlist = {}      # (b,sd) -> [D_g] group capacities, main then tail
    n_main = {}     # (b,sd) -> number of main groups
    membs = {}      # (c,b,sd) -> (jloc, take, g, p) arrays
    for b in range(NB):
        for sd in range(4):
            svals, sjls = [], []
            for c in range(NC):
                cc = cnt[c, b, sd]
                nz = np.nonzero(cc)[0]
                o = np.argsort(-cc[nz], kind="stable")
                svals.append(cc[nz][o])
                sjls.append(nz[o])
            gmax = max((len(v) + P - 1) // P for v in svals)
            prof = np.zeros((NC, gmax), np.int64)
            for c in range(NC):
                v = svals[c]
                r = np.arange(gmax) * P
                m = r < len(v)
                prof[c, m] = v[r[m]]
            base = np.maximum(np.floor(prof.mean(axis=0)).astype(np.int64), 1)
            # main memberships + leftovers
            t_vals, t_jls = [], []
            for c in range(NC):
                v, jl = svals[c], sjls[c]
                Dv = base[np.arange(len(v)) // P]
                take = np.minimum(v, Dv)
                left = v - take
                keep = left > 0
                t_vals.append(left[keep])
                t_jls.append(jl[keep])
            # tail: shared profile = max over cores of sorted leftover
            tso, tsj = [], []
            for c in range(NC):
                o = np.argsort(-t_vals[c], kind="stable")
                tso.append(t_vals[c][o])
                tsj.append(t_jls[c][o])
            tgmax = max((len(v) + P - 1) // P for v in tso) if any(
                len(v) for v in tso) else 0
            tprof = np.zeros(tgmax, np.int64)
            for g in range(tgmax):
                tprof[g] = max((v[g * P] if g * P < len(v) else 0)
                               for v in tso)
            Ds = list(base[:gmax]) + [int(x) for x in tprof]
            Dlist[(b, sd)] = Ds
            n_main[(b, sd)] = gmax
            for c in range(NC):
                v, jl = svals[c], sjls[c]
                Dv = base[np.arange(len(v)) // P]
                take_m = np.minimum(v, Dv)
                g_m = np.arange(len(v)) // P
                p_m = np.arange(len(v)) % P
                tv, tj = tso[c], tsj[c]
                g_t = gmax + np.arange(len(tv)) // P
                p_t = np.arange(len(tv)) % P
                membs[(c, b, sd)] = (
                    np.concatenate([jl, tj]),
                    np.concatenate([take_m, tv]),
                    np.concatenate([g_m, g_t]),
                    np.concatenate([p_m, p_t]))

    # ---- window packing: whole groups first-fit, grid cols compact
    # group sequence per bucket: for sd: main+tail groups in order
    buckets = []        # buckets[b] = list of window dicts
    colbase = {}        # (b, sd, g) -> global grid col of group start
    gcol = 0
    for b in range(NB):
        wins = []
        cur_segs = []    # (sd, g, c0_in_window, d)
        cur_used = 0

        def close_window():
            nonlocal cur_segs, cur_used, gcol
            if not cur_segs:
                return
            calls = []
            i = 0
            nm_tag = []
            for (sd, g, c0, d) in cur_segs:
                nm_tag.append((sd, g < n_main[(b, sd)]))
            while i < len(cur_segs):
                j = i
                while (j < len(cur_segs) and nm_tag[j] == nm_tag[i]
                       and j - i < SCALL):
                    j += 1
                calls.append((cur_segs[i][0], nm_tag[i][1], i, j - i))
                i = j
            wins.append(dict(cols=cur_used, segs=list(cur_segs), calls=calls))
            gcol += cur_used
            cur_segs, cur_used = [], 0

        for sd in range(4):
            for g, D in enumerate(Dlist[(b, sd)]):
                if D > WCOLS:
                    raise ValueError(f"group too wide: {D}")
                if cur_used + D > WCOLS:
                    close_window()
                colbase[(b, sd, g)] = ("pending", len(wins), cur_used)
                cur_segs.append((sd, g, cur_used, D))
                cur_used += D
        close_window()
        buckets.append(wins)
    # resolve colbase to global grid cols
    woff = {}
    gc = 0
    for b in range(NB):
        for w, win in enumerate(buckets[b]):
            woff[(b, w)] = gc
            gc += win["cols"]
    TOTCOL = gc
    for k, (_, w, c0) in list(colbase.items()):
        colbase[k] = woff[(k[0], w)] + c0

    # ---- edge -> slot assignment
    grid = np.full((NC, P, TOTCOL), DUMMY, np.int16)
    key_e = ((cd_e * NB + b_e) * 4 + sd_e) * SJ + jloc_e
    order_e = np.argsort(key_e, kind="stable")
    ks = key_e[order_e]
    starts = np.r_[0, np.nonzero(np.diff(ks))[0] + 1]
    run_start = np.zeros(len(ks), np.int64)
    run_start[starts] = starts
    run_start = np.maximum.accumulate(run_start)
    q_sorted = np.arange(len(ks)) - run_start

    e_ptr = 0
    eb = np.bincount(key_e // SJ, minlength=NC * NB * 4)
    for c in range(NC):
        for b in range(NB):
            for sd in range(4):
                n_e = eb[(c * NB + b) * 4 + sd]
                if n_e == 0:
                    continue
                sel = slice(e_ptr, e_ptr + n_e)
                e_ptr += n_e
                jl_m, tk_m, g_m, p_m = membs[(c, b, sd)]
                o = np.lexsort((g_m, jl_m))
                jl_s, tk_s, g_s, p_s = jl_m[o], tk_m[o], g_m[o], p_m[o]
                cum = np.cumsum(tk_s)
                jl_e2 = jloc_e[order_e[sel]]
                q_e = q_sorted[sel]
                cc = cnt[c, b, sd]
                cnt_off = np.zeros(SJ, np.int64)
                cnt_off[1:] = np.cumsum(cc)[:-1]
                e_pos = cnt_off[jl_e2] + q_e
                mi = np.searchsorted(cum, e_pos, side="right")
                colw = e_pos - (cum[mi] - tk_s[mi])
                cb = np.array([colbase[(b, sd, gg)]
                               for gg in range(len(Dlist[(b, sd)]))],
                              np.int64)
                gcol_e = cb[g_s[mi]] + colw
                grid[c, p_s[mi], gcol_e] = sloc_e[order_e[sel]]
    assert e_ptr == N_EDGES

    # ---- gidx per core (16-wrap is per-column-consistent, so any
    # column-aligned call boundary works without padding)
    gidx = np.empty((NC, P, TOTCOL * 8), np.int16)
    for c in range(NC):
        w16 = grid[c].T.reshape(-1).reshape(TOTCOL * 8, 16).T
        gidx[c] = np.tile(w16, (8, 1)).astype(np.int16)

    # ---- scatter ids per core in (b, w, call) order
    sidx_list = [[] for _ in range(NC)]
    sidx_off = []
    off = 0
    for b in range(NB):
        for w, win in enumerate(buckets[b]):
            for (sd, kind, i0, k) in win["calls"]:
                sidx_off.append(off)
                for c in range(NC):
                    jl_m, tk_m, g_m, p_m = membs[(c, b, sd)]
                    ids = np.full((k, P), TRASH, np.int64)
                    for s in range(k):
                        g = win["segs"][i0 + s][1]
                        m = g_m == g
                        ids[s, p_m[m]] = jl_m[m]
                    sidx_list[c].append(ids.reshape(-1))
                off += k * 8
    sidx = np.stack([_wrap16_blocks(sl) for sl in sidx_list])

    dpt = np.zeros((NC, P, CHUNK // P), np.float32)
    for c in range(NC):
        dchunk = np.zeros(CHUNK, np.float32)
        for sd in range(4):
            nsel = np.arange(SJ) + c * PER_CORE + sd * SJ
            dchunk[sd * STRIP:sd * STRIP + SJ] = dinv[nsel]
        dpt[c] = dchunk.reshape(CHUNK // P, P).T

    meta = dict(Dlist=Dlist, n_main=n_main, buckets=buckets, TOTCOL=TOTCOL,
                woff=woff, gidx_cols=TOTCOL * 8, sidx_cols=sidx.shape[2],
                sidx_off=sidx_off)
    return gidx, sidx, dpt, dinv, meta




class BassRunner:
    """Jit-once PJRT runner for a finalized bass module on 8 cores."""

    def __init__(self, nc, n_cores=8):
        import jax
        from jax.sharding import Mesh, PartitionSpec
        from jax.experimental.shard_map import shard_map
        import concourse.mybir as mybir
        from concourse import bass2jax
        from concourse.bass2jax import _bass_exec_p, partition_id_tensor

        bass2jax.install_neuronx_cc_hook()
        self.jax = jax
        self.nc = nc
        self.n_cores = n_cores
        partition_name = (nc.partition_id_tensor.name
                          if nc.partition_id_tensor else None)
        in_names, out_names, out_avals, zero_outs = [], [], [], []
        for alloc in nc.m.functions[0].allocations:
            if not isinstance(alloc, mybir.MemoryLocationSet):
                continue
            name = alloc.memorylocations[0].name
            if alloc.kind == "ExternalInput":
                if name != partition_name:
                    in_names.append(name)
            elif alloc.kind == "ExternalOutput":
                shape = tuple(alloc.tensor_shape)
                dtype = mybir.dt.np(alloc.dtype)
                out_avals.append(jax.core.ShapedArray(shape, dtype))
                out_names.append(name)
                zero_outs.append(np.zeros(shape, dtype))
        self.in_names = list(in_names)
        self.out_names = out_names
        self.out_avals = out_avals
        self.zero_outs = zero_outs
        n_params = len(self.in_names)
        n_outs = len(out_names)
        all_in_names = self.in_names + out_names
        if partition_name is not None:
            all_in_names.append(partition_name)

        def _body(*args):
            operands = list(args)
            if partition_name is not None:
                operands.append(partition_id_tensor())
            outs = _bass_exec_p.bind(
                *operands,
                out_avals=tuple(out_avals),
                in_names=tuple(all_in_names),
                out_names=tuple(out_names),
                lowering_input_output_aliases=(),
                sim_require_finite=True,
                sim_require_nnan=True,
                nc=nc,
            )
            return tuple(outs)

        devices = jax.devices()[:n_cores]
        self.mesh = Mesh(np.asarray(devices), ("core",))
        in_specs = (PartitionSpec("core"),) * (n_params + n_outs)
        out_specs = (PartitionSpec("core"),) * n_outs
        self.donate = tuple(range(n_params, n_params + n_outs))
        self.fn = jax.jit(
            shard_map(_body, mesh=self.mesh, in_specs=in_specs,
                      out_specs=out_specs, check_rep=False),
            donate_argnums=self.donate, keep_unused=True,
        )
        self.sharding = jax.sharding.NamedSharding(self.mesh,
                                                   PartitionSpec("core"))

    def put_inputs(self, in_maps):
        concat = []
        for name in self.in_names:
            arr = np.concatenate([np.asarray(m[name]) for m in in_maps], axis=0)
            concat.append(self.jax.device_put(arr, self.sharding))
        return concat

    def _zeros(self):
        return [self.jax.device_put(
                    np.zeros((self.n_cores * z.shape[0], *z.shape[1:]), z.dtype),
                    self.sharding)
                for z in self.zero_outs]

    def run(self, dev_inputs):
        outs = self.fn(*dev_inputs, *self._zeros())
        self.jax.block_until_ready(outs)
        return outs

    def time_runs(self, dev_inputs, n_rep=6):
        import time
        ts = []
        for _ in range(n_rep):
            zeros = self._zeros()
            self.jax.block_until_ready(zeros)
            t0 = time.monotonic()
            outs = self.fn(*dev_inputs, *zeros)
            self.jax.block_until_ready(outs)
            ts.append(time.monotonic() - t0)
        return min(ts), ts

    def results(self, outs):
        res = []
        for c in range(self.n_cores):
            d = {}
            for i, name in enumerate(self.out_names):
                d[name] = np.asarray(outs[i]).reshape(
                    self.n_cores, *self.out_avals[i].shape)[c]
            res.append(d)
        return res




NGRP = CHUNK // P  # 100

_runners = {}
_prep_cache = {}


def _build(meta):
    import concourse.bacc as bacc
    import concourse.tile as tile
    from concourse import mybir
    from concourse.masks import make_identity

    F = HID
    nc = bacc.Bacc(None, target_bir_lowering=False, num_devices=NC,
                   num_swdge_queues=4, dynamic_dma_scratch_size=2 ** 15)
    xT = nc.dram_tensor("xT", [F_IN, CHUNK], mybir.dt.float32,
                        kind="ExternalInput")
    w1 = nc.dram_tensor("w1", [F_IN, HID], mybir.dt.float32,
                        kind="ExternalInput")
    w2 = nc.dram_tensor("w2", [HID, N_CLS], mybir.dt.float32,
                        kind="ExternalInput")
    b1d = nc.dram_tensor("b1d", [P, HID], mybir.dt.float32,
                         kind="ExternalInput")
    b2d = nc.dram_tensor("b2d", [P, N_CLS], mybir.dt.float32,
                         kind="ExternalInput")
    dptd = nc.dram_tensor("dptd", [P, NGRP], mybir.dt.float32,
                          kind="ExternalInput")
    gidxd = nc.dram_tensor("gidxd", [P, meta["gidx_cols"]], mybir.dt.int16,
                           kind="ExternalInput")
    sidxd = nc.dram_tensor("sidxd", [P, meta["sidx_cols"]], mybir.dt.int16,
                           kind="ExternalInput")
    outd = nc.dram_tensor("outd", [CHUNK, N_CLS], mybir.dt.float32,
                          kind="ExternalOutput")

    agin1 = [nc.dram_tensor(f"agin1_{k}", [STRIP, F], mybir.dt.float32,
                            kind="Internal") for k in range(4)]
    agin2 = [nc.dram_tensor(f"agin2_{k}", [STRIP, F], mybir.dt.float32,
                            kind="Internal") for k in range(4)]
    table1 = [nc.dram_tensor(f"table1_{k}", [BUCKET_ROWS, F],
                             mybir.dt.float32, kind="Internal",
                             addr_space="Shared") for k in range(4)]
    table2 = [nc.dram_tensor(f"table2_{k}", [BUCKET_ROWS, F],
                             mybir.dt.float32, kind="Internal",
                             addr_space="Shared") for k in range(4)]
    yacc1 = [nc.dram_tensor(f"yacc1_{k}", [YROWS, F], mybir.dt.float32,
                            kind="Internal") for k in range(4)]
    yacc2 = [nc.dram_tensor(f"yacc2_{k}", [YROWS, F], mybir.dt.float32,
                            kind="Internal") for k in range(4)]

    buckets = meta["buckets"]
    sidx_off = meta["sidx_off"]
    qn = [0]

    with tile.TileContext(nc) as tc:
        with tc.tile_pool(name="c0", bufs=1) as cp, \
             tc.tile_pool(name="xs", bufs=2) as xsp, \
             tc.tile_pool(name="mm", bufs=4) as mmp, \
             tc.tile_pool(name="gw", bufs=4) as gwp, \
             tc.tile_pool(name="sb", bufs=4) as sbp, \
             tc.tile_pool(name="gi", bufs=4) as gip, \
             tc.tile_pool(name="ep", bufs=4) as epp, \
             tc.tile_pool(name="p1", bufs=4, space="PSUM") as pp1, \
             tc.tile_pool(name="p2", bufs=2, space="PSUM") as pp2:

            # ---- constants
            w1a = cp.tile([P, HID], mybir.dt.float32, tag="w1a")
            w1b = cp.tile([P, HID], mybir.dt.float32, tag="w1b")
            nc.sync.dma_start(out=w1a[:], in_=w1[0:P, :])
            nc.sync.dma_start(out=w1b[:], in_=w1[P:2 * P, :])
            w2t = cp.tile([HID, N_CLS], mybir.dt.float32, tag="w2t")
            nc.sync.dma_start(out=w2t[:], in_=w2[:])
            b1t = cp.tile([P, HID], mybir.dt.float32, tag="b1t")
            nc.sync.dma_start(out=b1t[:], in_=b1d[:])
            b2t = cp.tile([P, N_CLS], mybir.dt.float32, tag="b2t")
            nc.sync.dma_start(out=b2t[:], in_=b2d[:])
            dv = cp.tile([P, NGRP], mybir.dt.float32, tag="dv")
            nc.sync.dma_start(out=dv[:], in_=dptd[:])
            sit = cp.tile([P, meta["sidx_cols"]], mybir.dt.int16, tag="sit")
            nc.sync.dma_start(out=sit[:], in_=sidxd[:])
            ident = cp.tile([P, P], mybir.dt.float32, tag="id")
            make_identity(nc, ident[:])

            # ---- zero both layers' accumulators
            zt = cp.tile([P, YROWS * F // P], mybir.dt.float32, tag="zt")
            nc.vector.memset(zt[:], 0.0)
            for y in yacc1 + yacc2:
                nc.sync.dma_start(
                    out=y[:].flatten().rearrange("(p n) -> p n", p=P),
                    in_=zt[:])

            # ---- layer-1 matmul: h1 = (x @ W1) * dinv -> agin1 strips
            for ch in range(10):
                xa = xsp.tile([P, 1280], mybir.dt.float32, tag="xa")
                xb = xsp.tile([P, 1280], mybir.dt.float32, tag="xb")
                nc.sync.dma_start(out=xa[:],
                                  in_=xT[0:P, ch * 1280:(ch + 1) * 1280])
                nc.sync.dma_start(out=xb[:],
                                  in_=xT[P:2 * P, ch * 1280:(ch + 1) * 1280])
                for gl in range(10):
                    g = ch * 10 + gl
                    pst = pp1.tile([P, HID], mybir.dt.float32, tag="mmps")
                    nc.tensor.matmul(out=pst[:],
                                     lhsT=xa[:, gl * P:(gl + 1) * P],
                                     rhs=w1a[:], start=True, stop=False)
                    nc.tensor.matmul(out=pst[:],
                                     lhsT=xb[:, gl * P:(gl + 1) * P],
                                     rhs=w1b[:], start=False, stop=True)
                    ot = mmp.tile([P, HID], mybir.dt.float32, tag="ot")
                    nc.vector.tensor_tensor(
                        out=ot[:], in0=pst[:],
                        in1=dv[:, g:g + 1].to_broadcast([P, HID]),
                        op=mybir.AluOpType.mult)
                    sd, gl2 = g // 25, g % 25
                    nc.sync.dma_start(
                        out=agin1[sd][gl2 * P:(gl2 + 1) * P, :], in_=ot[:])
            for k in range(4):
                nc.gpsimd.collective_compute(
                    "AllGather", mybir.AluOpType.bypass,
                    replica_groups=[list(range(NC))],
                    ins=[agin1[k][:].opt()], outs=[table1[k][:].opt()])

            # ---- scatter-call bookkeeping for barriers / strip completion
            calls_seq = []
            for b in range(NB):
                for w, win in enumerate(buckets[b]):
                    for (sd, kind, i0, k) in win["calls"]:
                        calls_seq.append((b, sd, kind))
            last_any, last_main, tails = {}, {}, set()
            for i, (b, sd, kind) in enumerate(calls_seq):
                last_any[(b, sd)] = i
                if kind:
                    last_main[(b, sd)] = i
                else:
                    tails.add((b, sd))
            barrier_after = {}
            protected = set()
            for (b, sd) in tails:
                if (b, sd) in last_main:
                    barrier_after.setdefault(
                        last_main[(b, sd)], []).append(sd)
                    protected.add((b, sd, False))
            for (b, sd), i in last_any.items():
                if b + 1 < NB:
                    barrier_after.setdefault(i, []).append(sd)
                    protected.add((b + 1, sd, True))
            strip_done_at = {}
            for sd in range(4):
                i = max(last_any[(b, sd)] for b in range(NB)
                        if (b, sd) in last_any)
                strip_done_at[i] = sd

            # ---- aggregation (shared for both layers)
            # Scatters are deferred two windows behind the gather stream so
            # gather desc-gen never waits on reduce completion.  Barrier
            # reads (dummy RAW read of a yacc) are pre-issued right after
            # the last call of the sub-schedule they fence: concurrent CCE
            # RMW to one row loses updates (verified on HW), so any two
            # sub-schedules that can repeat a row are separated by a read.
            def aggregate(tables, yaccs, strip_cb):
                ci = [0]
                seen = set(protected)

                def barrier(sd):
                    brt = epp.tile([1, F], mybir.dt.float32, tag="bar")
                    nc.sync.dma_start(out=brt[:], in_=yaccs[sd][0:1, :])

                def emit_scatters(b, win, sbt):
                    for (sd, kind, i0, k) in win["calls"]:
                        soff = sidx_off[ci[0]]
                        key = (b, sd, kind)
                        if key not in seen:
                            seen.add(key)
                            if (b > 0) or (not kind):
                                barrier(sd)
                        nc.gpsimd.dma_scatter_add(
                            out_ap=yaccs[sd][:],
                            in_ap=sbt[:, i0:i0 + k, :],
                            idxs_ap=sit[:, soff:soff + k * 8],
                            num_idxs=k * P, num_idxs_reg=k * P,
                            elem_size=F, queue_num=qn[0] % 4)
                        qn[0] += 1
                        for sd2 in barrier_after.get(ci[0], ()):
                            barrier(sd2)
                        if ci[0] in strip_done_at:
                            strip_cb(strip_done_at[ci[0]])
                        ci[0] += 1

                pending = []
                for b in range(NB):
                    for w, win in enumerate(buckets[b]):
                        cols = win["cols"]
                        wc0 = meta["woff"][(b, w)]
                        git = gip.tile([P, WCOLS * 8], mybir.dt.int16,
                                       tag="git")
                        nc.scalar.dma_start(
                            out=git[:, :cols * 8],
                            in_=gidxd[:, wc0 * 8:(wc0 + cols) * 8])
                        gwt = gwp.tile([P, WCOLS, F], mybir.dt.float32,
                                       tag="gw")
                        for cc in range(0, cols, GCALL):
                            k8 = min(GCALL, cols - cc)
                            nc.gpsimd.dma_gather(
                                out_ap=gwt[:, cc:cc + k8, :],
                                in_ap=tables[b][:],
                                idxs_ap=git[:, cc * 8:(cc + k8) * 8],
                                num_idxs=k8 * P, num_idxs_reg=k8 * P,
                                elem_size=F, queue_num=qn[0] % 4)
                            qn[0] += 1
                        if len(pending) >= 2:
                            emit_scatters(*pending.pop(0))
                        sbt = sbp.tile([P, WCOLS, F], mybir.dt.float32,
                                       tag="sb")
                        for r, (sd, g, c0, d) in enumerate(win["segs"]):
                            nc.vector.tensor_reduce(
                                out=sbt[:, r, :],
                                in_=gwt[:, c0:c0 + d, :].rearrange(
                                    "p g f -> p f g"),
                                axis=mybir.AxisListType.X,
                                op=mybir.AluOpType.add)
                        pending.append((b, win, sbt))
                for p_ in pending:
                    emit_scatters(*p_)

            # ---- layer-1 epilogue strip: Rs = relu((Y+Town)*dinv+b1)*dinv
            def ep1_strip(sd):
                for gl in range(25):
                    g = sd * 25 + gl
                    dcol = dv[:, g:g + 1]
                    yt = epp.tile([P, F], mybir.dt.float32, tag="yt")
                    nc.sync.dma_start(out=yt[:],
                                      in_=yacc1[sd][gl * P:(gl + 1) * P, :])
                    tt = epp.tile([P, F], mybir.dt.float32, tag="tt")
                    nc.sync.dma_start(out=tt[:],
                                      in_=agin1[sd][gl * P:(gl + 1) * P, :])
                    t1 = epp.tile([P, F], mybir.dt.float32, tag="t1")
                    nc.vector.tensor_tensor(out=t1[:], in0=yt[:], in1=tt[:],
                                            op=mybir.AluOpType.add)
                    nc.vector.tensor_tensor(
                        out=t1[:], in0=t1[:],
                        in1=dcol.to_broadcast([P, F]),
                        op=mybir.AluOpType.mult)
                    nc.vector.tensor_tensor(out=t1[:], in0=t1[:], in1=b1t[:],
                                            op=mybir.AluOpType.add)
                    t2 = epp.tile([P, F], mybir.dt.float32, tag="t2")
                    nc.scalar.activation(
                        out=t2[:], in_=t1[:],
                        func=mybir.ActivationFunctionType.Relu)
                    nc.vector.tensor_tensor(
                        out=t2[:], in0=t2[:],
                        in1=dcol.to_broadcast([P, F]),
                        op=mybir.AluOpType.mult)
                    nc.sync.dma_start(
                        out=agin2[sd][gl * P:(gl + 1) * P, :], in_=t2[:])
                nc.gpsimd.collective_compute(
                    "AllGather", mybir.AluOpType.bypass,
                    replica_groups=[list(range(NC))],
                    ins=[agin2[sd][:].opt()], outs=[table2[sd][:].opt()])

            # ---- layer-2 epilogue strip: log_softmax((Y+Town)*dinv@W2+b2)
            # Exp per group, one Ln per strip over the batched row-sums
            # (avoids per-group activation-table reloads).
            lgbuf = cp.tile([P, NGRP * N_CLS], mybir.dt.float32, tag="lgb")
            smT = cp.tile([P, NGRP], mybir.dt.float32, tag="smT")
            lnT = cp.tile([P, NGRP], mybir.dt.float32, tag="lnT")

            def ep2_strip(sd):
                for gl in range(25):
                    g = sd * 25 + gl
                    dcol = dv[:, g:g + 1]
                    yt = epp.tile([P, F], mybir.dt.float32, tag="yt")
                    nc.sync.dma_start(out=yt[:],
                                      in_=yacc2[sd][gl * P:(gl + 1) * P, :])
                    tt = epp.tile([P, F], mybir.dt.float32, tag="tt")
                    nc.sync.dma_start(out=tt[:],
                                      in_=agin2[sd][gl * P:(gl + 1) * P, :])
                    t1 = epp.tile([P, F], mybir.dt.float32, tag="t1")
                    nc.vector.tensor_tensor(out=t1[:], in0=yt[:], in1=tt[:],
                                            op=mybir.AluOpType.add)
                    nc.vector.tensor_tensor(
                        out=t1[:], in0=t1[:],
                        in1=dcol.to_broadcast([P, F]),
                        op=mybir.AluOpType.mult)
                    pt = pp2.tile([HID, P], mybir.dt.float32, tag="pt")
                    nc.tensor.transpose(out=pt[:], in_=t1[:],
                                        identity=ident[:])
                    zt2 = epp.tile([HID, P], mybir.dt.float32, tag="zt2")
                    nc.vector.tensor_copy(out=zt2[:], in_=pt[:])
                    p2t = pp2.tile([P, N_CLS], mybir.dt.float32, tag="p2")
                    nc.tensor.matmul(out=p2t[:], lhsT=zt2[:], rhs=w2t[:],
                                     start=True, stop=True)
                    lg = lgbuf[:, g * N_CLS:(g + 1) * N_CLS]
                    nc.vector.tensor_tensor(out=lg, in0=p2t[:], in1=b2t[:],
                                            op=mybir.AluOpType.add)
                    mx = epp.tile([P, 1], mybir.dt.float32, tag="mx")
                    nc.vector.tensor_reduce(out=mx[:], in_=lg,
                                            axis=mybir.AxisListType.X,
                                            op=mybir.AluOpType.max)
                    nc.vector.tensor_tensor(
                        out=lg, in0=lg,
                        in1=mx[:].to_broadcast([P, N_CLS]),
                        op=mybir.AluOpType.subtract)
                    ex = epp.tile([P, N_CLS], mybir.dt.float32, tag="ex")
                    nc.scalar.activation(
                        out=ex[:], in_=lg,
                        func=mybir.ActivationFunctionType.Exp)
                    nc.vector.tensor_reduce(out=smT[:, g:g + 1], in_=ex[:],
                                            axis=mybir.AxisListType.X,
                                            op=mybir.AluOpType.add)
                # one Ln per strip, then write the strip's outputs
                nc.scalar.activation(
                    out=lnT[:, sd * 25:(sd + 1) * 25],
                    in_=smT[:, sd * 25:(sd + 1) * 25],
                    func=mybir.ActivationFunctionType.Ln)
                for gl in range(25):
                    g = sd * 25 + gl
                    og = epp.tile([P, N_CLS], mybir.dt.float32, tag="og")
                    nc.vector.tensor_tensor(
                        out=og[:], in0=lgbuf[:, g * N_CLS:(g + 1) * N_CLS],
                        in1=lnT[:, g:g + 1].to_broadcast([P, N_CLS]),
                        op=mybir.AluOpType.subtract)
                    nc.sync.dma_start(
                        out=outd[(sd * STRIP + gl * P):
                                 (sd * STRIP + (gl + 1) * P), :],
                        in_=og[:])

            aggregate(table1, yacc1, ep1_strip)
            aggregate(table2, yacc2, ep2_strip)
    nc.finalize()
    return nc


def _prep(edge_index):
    key = hashlib.sha1(np.ascontiguousarray(edge_index).tobytes()).hexdigest()
    if key not in _prep_cache:
        _prep_cache[key] = build_schedule(edge_index)
    return _prep_cache[key]


def _meta_key(meta):
    h = hashlib.sha1()
    h.update(repr([meta["TOTCOL"], sorted(meta["woff"].items()),
                   [(b, w["cols"], tuple(w["segs"]), tuple(w["calls"]))
                    for b, bw in enumerate(meta["buckets"])
                    for w in bw]]).encode())
    return h.hexdigest()


def get_runner(meta):
    key = _meta_key(meta)
    if key not in _runners:
        _runners[key] = BassRunner(_build(meta), NC)
    return _runners[key]


def make_in_maps(x, W1, b1, W2, b2, gidx, sidx, dpt):
    x = np.asarray(x, np.float32)
    b1r = np.tile(np.asarray(b1, np.float32)[None, :], (P, 1))
    b2r = np.tile(np.asarray(b2, np.float32)[None, :], (P, 1))
    in_maps = []
    for c in range(NC):
        xs = np.zeros((F_IN, CHUNK), np.float32)
        for sd in range(4):
            n0 = c * PER_CORE + sd * SJ
            xs[:, sd * STRIP:sd * STRIP + SJ] = x[n0:n0 + SJ].T
        in_maps.append({
            "xT": xs, "w1": np.asarray(W1, np.float32),
            "w2": np.asarray(W2, np.float32), "b1d": b1r, "b2d": b2r,
            "dptd": dpt[c], "gidxd": gidx[c], "sidxd": sidx[c]})
    return in_maps


def assemble_out(res):
    out = np.empty((N_NODES, N_CLS), np.float32)
    for c in range(NC):
        oc = res[c]["outd"]
        for sd in range(4):
            n0 = c * PER_CORE + sd * SJ
            out[n0:n0 + SJ] = oc[sd * STRIP:sd * STRIP + SJ]
    return out


def kernel(x, edge_index, pos_edge_index, neg_edge_index, masked_nodes,
           W1, b1, W2, b2):
    gidx, sidx, dpt, dinv, meta = _prep(np.asarray(edge_index))
    runner = get_runner(meta)
    in_maps = make_in_maps(x, W1, b1, W2, b2, gidx, sidx, dpt)
    res = runner.results(runner.run(runner.put_inputs(in_maps)))
    return assemble_out(res)

